# revision 14
# baseline (speedup 1.0000x reference)
"""Trainium2 Bass kernel for nn_MultiHeadAttention_37477884625313.

Multi-head attention (B=4, C=512, T=1024, H=8, d=64) with the reference's
relative-position terms (corner score entries + the +/-4 attn band x emb_v).

Sharding: 8 cores = 4 batches x 2 query-halves; k/v host-rotated by -i0 so
the SPMD program is identical across cores.

Schedule: head-pair-major pipeline.  The Act queue carries ONLY exp (the
~39us serial floor).  Scores stream pair-major; the PE interleaves deferred
projection chains and ctx matmul chunks between score blocks so it never
blocks the exp stream.  eT transposes alternate SP/DVE queues; band staging
rides GpSimd (SWDGE) except the last pair (SP, for tail latency).  Band and
Z paths drain into SBUF so no psum tile is held across the tail.
"""

import sys

sys.path.insert(0, "/opt/trn_rl_repo")

import numpy as np
import ml_dtypes

import concourse.bass as bass
import concourse.mybir as mybir
import concourse.tile as tile
from concourse import bacc
from concourse.ap import AP
from concourse.masks import make_identity
from concourse.bass_utils import run_bass_kernel_spmd

BF16 = ml_dtypes.bfloat16
P = 128
C = 512
T = 1024
H = 8
D = 64
TQ = 512          # queries per core
IB = 4            # i-blocks per core
NCT = 4           # channel tiles (512 / 128)
W = 136           # band window width (128 + 8)
EXP = mybir.ActivationFunctionType.Exp
IDENT = mybir.ActivationFunctionType.Identity

_CACHE = {}


def build_nc():
    nc = bacc.Bacc("TRN2", target_bir_lowering=False)
    f32, bf = mybir.dt.float32, mybir.dt.bfloat16
    add_op = mybir.AluOpType.add
    mul_op = mybir.AluOpType.mult

    xq = nc.declare_dram_parameter("xq", [P, NCT, TQ], bf, isOutput=False)
    xk = nc.declare_dram_parameter("xk", [P, NCT, T], bf, isOutput=False)
    xv = nc.declare_dram_parameter("xv", [P, NCT, T], bf, isOutput=False)
    wqT = nc.declare_dram_parameter("wqT", [P, NCT, C], bf, isOutput=False)
    wkT = nc.declare_dram_parameter("wkT", [P, NCT, C], bf, isOutput=False)
    wvT = nc.declare_dram_parameter("wvT", [P, NCT, C], bf, isOutput=False)
    woT = nc.declare_dram_parameter("woT", [P, NCT, C], bf, isOutput=False)
    bq8 = nc.declare_dram_parameter("bq8", [P, NCT], f32, isOutput=False)
    bod = nc.declare_dram_parameter("bod", [P, NCT], f32, isOutput=False)
    evpr = nc.declare_dram_parameter("evpr", [P, NCT, P], bf, isOutput=False)
    oneh = nc.declare_dram_parameter("oneh", [8, NCT, P], bf, isOutput=False)
    cor1d = nc.declare_dram_parameter("cor1d", [P, H, 5], bf, isOutput=False)
    cor2d = nc.declare_dram_parameter("cor2d", [P, H, 3], bf, isOutput=False)
    amask = nc.declare_dram_parameter("amask", [P, IB, P], bf, isOutput=False)
    Out = nc.declare_dram_parameter("Out", [NCT, P, TQ], f32, isOutput=True)
    import os
    DBG = bool(os.environ.get("KDBG"))
    if DBG:
        DQ = nc.declare_dram_parameter("DQ", [P, NCT, TQ], mybir.dt.bfloat16, isOutput=True)
        DK = nc.declare_dram_parameter("DK", [P, NCT, T], mybir.dt.bfloat16, isOutput=True)
        DV = nc.declare_dram_parameter("DV", [P, 8, C], mybir.dt.bfloat16, isOutput=True)
        DZ = nc.declare_dram_parameter("DZ", [8, TQ], mybir.dt.bfloat16, isOutput=True)
        DCT = nc.declare_dram_parameter("DCT", [P, NCT, TQ], mybir.dt.bfloat16, isOutput=True)
        DBM = nc.declare_dram_parameter("DBM", [P, NCT, TQ], mybir.dt.float32, isOutput=True)
        DAB = nc.declare_dram_parameter("DAB", [P, IB, P], mybir.dt.bfloat16, isOutput=True)
        DET = nc.declare_dram_parameter("DET", [P, H, 8, IB, P], mybir.dt.bfloat16, isOutput=True)

    stg = [[nc.dram_tensor(f"stg{ib}_{a}", [P, 2, W], bf)
            for a in range(NCT)] for ib in range(IB)]

    with tile.TileContext(nc) as tc:
        with (
            tc.tile_pool(name="persist", bufs=1) as pp,
            tc.tile_pool(name="epool", bufs=3) as ep,
            tc.tile_pool(name="psS", bufs=2, space="PSUM") as psS,
            tc.tile_pool(name="psC", bufs=2, space="PSUM") as psC,
            tc.tile_pool(name="psT", bufs=2, space="PSUM") as psT,
        ):
            # ---- persistent SBUF tiles --------------------------------
            xq_sb = pp.tile([P, NCT, TQ], bf, name="xq_sb")
            xk_sb = pp.tile([P, NCT, T], bf, name="xk_sb")
            xv_sb = pp.tile([P, NCT, T], bf, name="xv_sb")
            wq_sb = pp.tile([P, NCT, C], bf, name="wq_sb")
            wk_sb = pp.tile([P, NCT, C], bf, name="wk_sb")
            wv_sb = pp.tile([P, NCT, C], bf, name="wv_sb")
            wo_sb = pp.tile([P, NCT, C], bf, name="wo_sb")
            bq_sb = pp.tile([P, NCT], f32, name="bq_sb")
            bo_sb = pp.tile([P, NCT], f32, name="bo_sb")
            ev_sb = pp.tile([P, NCT, P], bf, name="ev_sb")
            oh_sb = pp.tile([8, NCT, P], bf, name="oh_sb")
            cor1 = pp.tile([P, H, 5], bf, name="cor1")
            cor2 = pp.tile([P, H, 3], bf, name="cor2")
            am_sb = pp.tile([P, IB, P], bf, name="am_sb")

            q_sb = pp.tile([P, NCT, TQ], bf, name="q_sb")
            k_sb = pp.tile([P, NCT, T], bf, name="k_sb")
            vT_sb = pp.tile([P, 8, C], bf, name="vT_sb")
            eT_sb = pp.tile([P, H, 8, IB, P], bf, name="eT_sb")
            zall = pp.tile([P, IB, H], f32, name="zall")
            zrec = pp.tile([P, IB, H], bf, name="zrec")
            recz = pp.tile([8, TQ], bf, name="recz")
            rzb = pp.tile([P, NCT, TQ], f32, name="rzb")
            abg = pp.tile([P, IB, H, 16], bf, name="abg")
            abT_all = pp.tile([P, IB, P], bf, name="abT_all")
            bandm = pp.tile([P, NCT, TQ], f32, name="bandm")
            ctxm = pp.tile([P, 2, TQ], f32, name="ctxm")
            ctxn = pp.tile([P, NCT, TQ], bf, name="ctxn")

            # ---- input loads (use order; SP queue) --------------------
            nc.sync.dma_start(wq_sb[:], wqT[:])
            nc.sync.dma_start(xq_sb[:], xq[:])
            nc.sync.dma_start(bq_sb[:], bq8[:])
            nc.sync.dma_start(wk_sb[:], wkT[:])
            nc.sync.dma_start(xk_sb[:], xk[:])
            nc.sync.dma_start(cor1[:], cor1d[:])
            nc.sync.dma_start(cor2[:], cor2d[:])
            nc.sync.dma_start(wv_sb[:], wvT[:])
            nc.sync.dma_start(xv_sb[:], xv[:])
            nc.sync.dma_start(am_sb[:], amask[:])
            nc.sync.dma_start(ev_sb[:], evpr[:])
            nc.sync.dma_start(oh_sb[:], oneh[:])
            nc.sync.dma_start(wo_sb[:], woT[:])
            nc.sync.dma_start(bo_sb[:], bod[:])

            idbf = pp.tile([P, P], bf, name="idbf")
            make_identity(nc, idbf[:])
            nc.vector.memset(abg[:], 0.0)

            # ---- projection chains ------------------------------------
            def q_proj(co):
                pq = psC.tile([P, TQ], f32, name="pq")
                for ci in range(NCT):
                    nc.tensor.matmul(pq, wq_sb[:, ci, co * P:(co + 1) * P],
                                     xq_sb[:, ci, :], start=(ci == 0), stop=(ci == 3))
                nc.vector.tensor_scalar_add(q_sb[:, co, :], pq, bq_sb[:, co:co + 1])

            def k_proj_half(co, nh):
                # one j-half of the K projection for channel block co
                pk = psC.tile([P, TQ], f32, name="pq")
                ns = slice(nh * 512, nh * 512 + 512)
                for ci in range(NCT):
                    nc.tensor.matmul(pk, wk_sb[:, ci, co * P:(co + 1) * P],
                                     xk_sb[:, ci, ns],
                                     start=(ci == 0), stop=(ci == 3))
                nc.vector.tensor_copy(k_sb[:, co, ns], pk)

            def v_proj(tb):
                pv = psC.tile([P, TQ], f32, name="pq")
                for ci in range(NCT):
                    nc.tensor.matmul(pv, xv_sb[:, ci, tb * P:(tb + 1) * P],
                                     wv_sb[:, ci, :], start=(ci == 0), stop=(ci == 3))
                nc.vector.tensor_copy(vT_sb[:, tb, :], pv)

            pcs = {}

            def ctx_chunk(a, hh, jbs):
                # 4 accumulation matmuls into pc(a); creates the tile on
                # first use, stops on the very last (hh==1, jb==7)
                if a not in pcs:
                    pcs[a] = psC.tile([P, TQ], f32, name="pq")
                pc = pcs[a]
                h = 2 * a + hh
                for jb in jbs:
                    nc.tensor.matmul(
                        pc[hh * D:hh * D + D, :],
                        vT_sb[:, jb, h * D:h * D + D],
                        eT_sb[:, h, jb, :, :],
                        start=(jb == 0),
                        stop=(a <= 1 and jb == 7),
                        tile_position=(0, hh * D),
                        skip_group_check=True,
                    )
                if a <= 1 and hh == 1 and jbs[-1] == 7:
                    # spill to SBUF so the psC buffer recycles for a+2;
                    # a>=2 tiles stay open: band matmuls stop them
                    nc.vector.tensor_copy(ctxm[:, a, :], pc[:])

            # deferred PE work: (min_block_index, fn), drained in FIFO
            # order once the block counter passes min_block_index
            fill_q = [(0, lambda: k_proj_half(1, 0)),
                      (0, lambda: k_proj_half(1, 1)),
                      (0, lambda: q_proj(1))]
            for tb in range(8):
                fill_q.append((0, lambda t=tb: v_proj(t)))
            for co in range(2, NCT):
                fill_q.append((0, lambda c=co: k_proj_half(c, 0)))
                fill_q.append((0, lambda c=co: k_proj_half(c, 1)))
                fill_q.append((0, lambda c=co: q_proj(c)))

            def drain(bi):
                # pop at most one eligible unit per score block
                if fill_q and fill_q[0][0] <= bi:
                    fill_q.pop(0)[1]()

            # ---- pair-major scores / exp / transpose / stage ----------
            q_proj(0)
            k_proj_half(0, 0)
            k_proj_half(0, 1)


            bi = 0
            for a in range(NCT):
                if a >= 1:
                    # ctx chunks of the previous pair: hold until the
                    # pair's transposes have certainly landed
                    for ci, (hh, j) in enumerate(
                            [(0, (0, 1, 2, 3)), (0, (4, 5, 6, 7)),
                             (1, (0, 1, 2, 3)), (1, (4, 5, 6, 7))]):
                        fill_q.append((a * 8 + 3 + ci,
                                       lambda aa=a - 1, hhh=hh, jj=j:
                                       ctx_chunk(aa, hhh, jj)))
                for ib in ((3, 0, 1, 2) if a == 3 else range(IB)):
                    isl = slice(ib * P, ib * P + P)
                    e_t = ep.tile([P, 2, T], bf, name="e_t")
                    for hh in range(2):
                        h = 2 * a + hh
                        hp = hh * D
                        sc = psS.tile([P, T], f32, name="mm")
                        for nh in range(2):
                            ns = slice(nh * 512, nh * 512 + 512)
                            nc.tensor.matmul(sc[:, ns],
                                             q_sb[hp:hp + D, a, isl],
                                             k_sb[hp:hp + D, a, ns],
                                             start=True, stop=True)
                        if ib == 0:
                            nc.vector.tensor_tensor(out=sc[0:32, 1019:1024],
                                                    in0=sc[0:32, 1019:1024],
                                                    in1=cor1[0:32, h, :],
                                                    op=add_op)
                        if ib == 3:
                            nc.vector.tensor_tensor(out=sc[96:128, 512:515],
                                                    in0=sc[96:128, 512:515],
                                                    in1=cor2[96:128, h, :],
                                                    op=add_op)
                        nc.scalar.activation(e_t[:, hh, :], sc[:], EXP,
                                             accum_out=zall[:, ib, h:h + 1])
                        drain(bi)
                        bi += 1
                    # one merged transpose for both heads of the pair
                    nc.sync.dma_start_transpose(
                        eT_sb[:, 2 * a:2 * a + 2, :, ib, :], e_t[:, :, :])

                    # stage band window to DRAM; last pair rides SP
                    hsl = slice(2 * a, 2 * a + 2)
                    st = stg[ib][a]
                    if ib == 0:
                        nc.gpsimd.dma_start(st[:, :, 4:W], e_t[:, :, 0:W - 4])
                        nc.gpsimd.dma_start(st[:, :, 0:4], e_t[:, :, T - 4:T])
                    else:
                        lo = ib * P - 4
                        nc.gpsimd.dma_start(st[:, :, :], e_t[:, :, lo:lo + W])
                    diag = AP(tensor=st[:].tensor, offset=0,
                              ap=[[2 * W + 1, P], [W, 2], [1, 9]])
                    nc.gpsimd.dma_start(abg[:, ib, hsl, 0:9], diag)

            while fill_q:
                fill_q.pop(0)[1]()

            # ---- tail: Z transposes, band, ctx(3) ---------------------
            def z_tr(ib):
                with nc.allow_low_precision(reason="1/Z in bf16"):
                    nc.vector.reciprocal(zrec[:, ib, :], zall[:, ib, :])
                pzt = psT.tile([P, P], bf, name="tp")[0:8, :]
                nc.tensor.transpose(pzt, zrec[:, ib, :], idbf[:])
                nc.vector.tensor_copy(recz[:, ib * P:(ib + 1) * P], pzt)

            def ab_tr(ib):
                abm = ep.tile([P, P], bf, name="abm")
                nc.vector.tensor_tensor(
                    out=abm[:],
                    in0=abg[:, ib, :, :].rearrange("p h x -> p (h x)"),
                    in1=am_sb[:, ib, :], op=mul_op)
                pabt = psT.tile([P, P], bf, name="tp")
                nc.tensor.transpose(pabt[:], abm[:], idbf[:])
                nc.vector.tensor_copy(abT_all[:, ib, :], pabt[:])

            z_tr(3)
            z_tr(0)
            ab_tr(3)
            z_tr(1)
            ab_tr(0)
            z_tr(2)
            ctx_chunk(3, 0, (0, 1, 2, 3))
            ctx_chunk(3, 0, (4, 5, 6, 7))
            ab_tr(1)
            ctx_chunk(3, 1, (0, 1, 2, 3))
            ab_tr(2)
            ctx_chunk(3, 1, (4, 5, 6, 7))

            # 1/Z row broadcast -> [128, TQ] per pair (PE, then SBUF)
            for a in range(NCT):
                pz = psT.tile([P, TQ], f32, name="tp")
                nc.tensor.matmul(pz[:], oh_sb[:, a, :], recz[:],
                                 start=True, stop=True)
                nc.vector.tensor_copy(rzb[:, a, :], pz[:])

            # band matmuls: pairs 0/1 via transient psum + SBUF spill,
            # pairs 2/3 accumulate into the still-open ctx psum
            for a in (2, 3, 0, 1):
                if a >= 2:
                    for ib in range(IB):
                        nc.tensor.matmul(pcs[a][:, ib * P:(ib + 1) * P],
                                         ev_sb[:, a, :], abT_all[:, ib, :],
                                         start=False, stop=True,
                                         skip_group_check=True)
                else:
                    pba = psT.tile([P, TQ], f32, name="tp")
                    for ib in range(IB):
                        nc.tensor.matmul(pba[:, ib * P:(ib + 1) * P],
                                         ev_sb[:, a, :], abT_all[:, ib, :],
                                         start=True, stop=True,
                                         skip_group_check=True)
                    nc.vector.tensor_copy(bandm[:, a, :], pba[:])

            # ---- normalize ------------------------------------------
            for a in (2, 3, 0, 1):
                if a >= 2:
                    nc.vector.tensor_tensor(out=ctxn[:, a, :],
                                            in0=pcs[a][:],
                                            in1=rzb[:, a, :], op=mul_op)
                else:
                    csum = ep.tile([P, TQ], f32, name="csum")
                    nc.vector.tensor_tensor(out=csum[:], in0=ctxm[:, a, :],
                                            in1=bandm[:, a, :], op=add_op)
                    nc.vector.tensor_tensor(out=ctxn[:, a, :], in0=csum[:],
                                            in1=rzb[:, a, :], op=mul_op)

            # ---- output projection (+bias via Act copy) --------------
            for co in range(NCT):
                po = psS.tile([P, T], f32, name="mm")[:, 0:TQ]
                for ci in range(NCT):
                    nc.tensor.matmul(po, wo_sb[:, ci, co * P:(co + 1) * P],
                                     ctxn[:, ci, :], start=(ci == 0), stop=(ci == 3))
                o_sb = ep.tile([P, TQ], f32, name="o_sb")
                nc.scalar.activation(o_sb[:], po, IDENT,
                                     bias=bo_sb[:, co:co + 1])
                nc.sync.dma_start(Out[co], o_sb[:])

            if DBG:
                nc.sync.dma_start(DQ[:], q_sb[:])
                nc.sync.dma_start(DK[:], k_sb[:])
                nc.sync.dma_start(DV[:], vT_sb[:])
                nc.sync.dma_start(DZ[:], recz[:])
                nc.sync.dma_start(DCT[:], ctxn[:])
                nc.sync.dma_start(DBM[:], bandm[:])
                nc.sync.dma_start(DAB[:], abT_all[:])
                nc.sync.dma_start(DET[:], eT_sb[:])

    nc.compile()
    return nc


def _prep(inputs):
    x_q = np.asarray(inputs["x_q"], np.float32)
    x_k = np.asarray(inputs["x_k"], np.float32)
    x_v = np.asarray(inputs["x_v"], np.float32)
    Wq = np.asarray(inputs["Wq"], np.float32)
    Wk = np.asarray(inputs["Wk"], np.float32)
    Wv = np.asarray(inputs["Wv"], np.float32)
    Wo = np.asarray(inputs["Wo"], np.float32)
    bq = np.asarray(inputs["bq"], np.float32)
    bo = np.asarray(inputs["bo"], np.float32)
    bv = np.asarray(inputs["bv"], np.float32)
    ek = np.asarray(inputs["emb_rel_k"], np.float32)
    ev = np.asarray(inputs["emb_rel_v"], np.float32)

    def ctile(a):  # (C, X) -> (P, NCT, X) partition-first
        return np.ascontiguousarray(a.reshape(NCT, P, -1).transpose(1, 0, 2))

    wqT = ctile(Wq.T * 0.125).astype(BF16)
    wkT = ctile(Wk.T).astype(BF16)
    wvT = ctile(Wv.T).astype(BF16)
    woT = ctile(Wo.T).astype(BF16)
    bq8 = np.ascontiguousarray((bq * 0.125).reshape(NCT, P).T).astype(np.float32)
    bo_eff = bo + Wo @ bv
    bod = np.ascontiguousarray(bo_eff.reshape(NCT, P).T).astype(np.float32)

    evpr = np.zeros((NCT, P, P), np.float32)
    for a in range(NCT):
        for hh in range(2):
            h = 2 * a + hh
            for mt in range(9):
                evpr[a, 32 * a + 16 * hh + mt, hh * D:(hh + 1) * D] = ev[h, mt]
    evpr = np.ascontiguousarray(evpr.transpose(1, 0, 2)).astype(BF16)

    oneh = np.zeros((NCT, 8, P), np.float32)
    for a in range(NCT):
        for cc in range(P):
            oneh[a, 2 * a + cc // D, cc] = 1.0
    oneh = np.ascontiguousarray(oneh.transpose(1, 0, 2)).astype(BF16)

    in_maps = []
    for core in range(8):
        b, half = core // 2, core % 2
        i0 = half * TQ
        cor1 = np.zeros((P, H, 5), np.float32)
        cor2 = np.zeros((P, H, 3), np.float32)
        if half == 0:
            qc = (Wq @ x_q[b][:, 0:5]) * 0.125 + (bq[:, None] * 0.125)
            for h in range(H):
                for p in range(5):
                    for c in range(p, 5):
                        cor1[p, h, c] = qc[h * D:(h + 1) * D, p] @ ek[h, c - p]
        else:
            qc = (Wq @ x_q[b][:, 1019:1023]) * 0.125 + (bq[:, None] * 0.125)
            for h in range(H):
                for p in (125, 126, 127):
                    for c in range(3):
                        m = 133 + c - p
                        if 6 <= m <= 8:
                            cor2[p, h, c] = qc[h * D:(h + 1) * D, (895 + p) - 1019] @ ek[h, m]
        amask = np.zeros((IB, P, P), np.float32)
        for ib in range(IB):
            pvec = np.arange(P)
            for a in range(NCT):
                for mt in range(9):
                    j_g = i0 + ib * P + pvec + mt - 4
                    ok = ((j_g >= 0) & (j_g < T)).astype(np.float32)
                    amask[ib, :, a * 32 + 0 * 16 + mt] = ok
                    amask[ib, :, a * 32 + 1 * 16 + mt] = ok
        amask = np.ascontiguousarray(amask.transpose(1, 0, 2))
        in_maps.append({
            "xq": ctile(x_q[b][:, i0:i0 + TQ]).astype(BF16),
            "xk": ctile(np.roll(x_k[b], -i0, axis=1)).astype(BF16),
            "xv": ctile(np.roll(x_v[b], -i0, axis=1)).astype(BF16),
            "wqT": wqT, "wkT": wkT, "wvT": wvT, "woT": woT,
            "bq8": bq8, "bod": bod, "evpr": evpr, "oneh": oneh,
            "cor1d": cor1.astype(BF16), "cor2d": cor2.astype(BF16),
            "amask": amask.astype(BF16),
        })
    return in_maps


def kernel(**inputs):
    if "nc" not in _CACHE:
        _CACHE["nc"] = build_nc()
    nc = _CACHE["nc"]
    in_maps = _prep(inputs)
    res = run_bass_kernel_spmd(nc, in_maps, list(range(8)))
    out = np.zeros((4, C, T), np.float32)
    for core in range(8):
        b, half = core // 2, core % 2
        o = np.asarray(res.results[core]["Out"]).reshape(C, TQ)
        out[b][:, half * TQ:(half + 1) * TQ] = o
    return out


# revision 15
# speedup vs baseline: 1.0581x; 1.0581x over previous
"""Trainium2 Bass kernel for nn_MultiHeadAttention_37477884625313.

Multi-head attention (B=4, C=512, T=1024, H=8, d=64) with the reference's
relative-position terms (corner score entries + the +/-4 attn band x emb_v).

Sharding: 8 cores = 4 batches x 2 query-halves; k/v host-rotated by -i0 so
the SPMD program is identical across cores.

Schedule: head-pair-major pipeline.  The Act queue carries ONLY exp (the
~39us serial floor).  Scores stream pair-major; the PE interleaves deferred
projection chains and ctx matmul chunks between score blocks so it never
blocks the exp stream.  eT transposes alternate SP/DVE queues; band staging
rides GpSimd (SWDGE) except the last pair (SP, for tail latency).  Band and
Z paths drain into SBUF so no psum tile is held across the tail.
"""

import sys

sys.path.insert(0, "/opt/trn_rl_repo")

import numpy as np
import ml_dtypes

import concourse.bass as bass
import concourse.mybir as mybir
import concourse.tile as tile
from concourse import bacc
from concourse.ap import AP
from concourse.masks import make_identity
from concourse.bass_utils import run_bass_kernel_spmd

BF16 = ml_dtypes.bfloat16
P = 128
C = 512
T = 1024
H = 8
D = 64
TQ = 512          # queries per core
IB = 4            # i-blocks per core
NCT = 4           # channel tiles (512 / 128)
W = 136           # band window width (128 + 8)
EXP = mybir.ActivationFunctionType.Exp
IDENT = mybir.ActivationFunctionType.Identity

_CACHE = {}


def build_nc():
    nc = bacc.Bacc("TRN2", target_bir_lowering=False)
    f32, bf = mybir.dt.float32, mybir.dt.bfloat16
    add_op = mybir.AluOpType.add
    mul_op = mybir.AluOpType.mult

    xq = nc.declare_dram_parameter("xq", [P, NCT, TQ], bf, isOutput=False)
    xk = nc.declare_dram_parameter("xk", [P, NCT, T], bf, isOutput=False)
    xv = nc.declare_dram_parameter("xv", [P, NCT, T], bf, isOutput=False)
    wqT = nc.declare_dram_parameter("wqT", [P, NCT, C], bf, isOutput=False)
    wkT = nc.declare_dram_parameter("wkT", [P, NCT, C], bf, isOutput=False)
    wvT = nc.declare_dram_parameter("wvT", [P, NCT, C], bf, isOutput=False)
    woT = nc.declare_dram_parameter("woT", [P, NCT, C], bf, isOutput=False)
    bq8 = nc.declare_dram_parameter("bq8", [P, NCT], f32, isOutput=False)
    bod = nc.declare_dram_parameter("bod", [P, NCT], f32, isOutput=False)
    evpr = nc.declare_dram_parameter("evpr", [P, NCT, P], bf, isOutput=False)
    oneh = nc.declare_dram_parameter("oneh", [8, NCT, P], bf, isOutput=False)
    cor1d = nc.declare_dram_parameter("cor1d", [P, H, 5], bf, isOutput=False)
    cor2d = nc.declare_dram_parameter("cor2d", [P, H, 3], bf, isOutput=False)
    amask = nc.declare_dram_parameter("amask", [P, IB, P], bf, isOutput=False)
    Out = nc.declare_dram_parameter("Out", [NCT, P, TQ], f32, isOutput=True)
    import os
    DBG = bool(os.environ.get("KDBG"))
    if DBG:
        DQ = nc.declare_dram_parameter("DQ", [P, NCT, TQ], mybir.dt.bfloat16, isOutput=True)
        DK = nc.declare_dram_parameter("DK", [P, NCT, T], mybir.dt.bfloat16, isOutput=True)
        DV = nc.declare_dram_parameter("DV", [P, 8, C], mybir.dt.bfloat16, isOutput=True)
        DZ = nc.declare_dram_parameter("DZ", [8, TQ], mybir.dt.bfloat16, isOutput=True)
        DCT = nc.declare_dram_parameter("DCT", [P, NCT, TQ], mybir.dt.bfloat16, isOutput=True)
        DBM = nc.declare_dram_parameter("DBM", [P, NCT, TQ], mybir.dt.float32, isOutput=True)
        DAB = nc.declare_dram_parameter("DAB", [P, IB, P], mybir.dt.bfloat16, isOutput=True)
        DET = nc.declare_dram_parameter("DET", [P, H, 8, IB, P], mybir.dt.bfloat16, isOutput=True)

    stg = [[nc.dram_tensor(f"stg{ib}_{a}", [P, 2, W], bf)
            for a in range(NCT)] for ib in range(IB)]

    with tile.TileContext(nc) as tc:
        with (
            tc.tile_pool(name="persist", bufs=1) as pp,
            tc.tile_pool(name="epool", bufs=3) as ep,
            tc.tile_pool(name="psS", bufs=2, space="PSUM") as psS,
            tc.tile_pool(name="psC", bufs=2, space="PSUM") as psC,
            tc.tile_pool(name="psT", bufs=2, space="PSUM") as psT,
        ):
            # ---- persistent SBUF tiles --------------------------------
            xq_sb = pp.tile([P, NCT, TQ], bf, name="xq_sb")
            xk_sb = pp.tile([P, NCT, T], bf, name="xk_sb")
            xv_sb = pp.tile([P, NCT, T], bf, name="xv_sb")
            wq_sb = pp.tile([P, NCT, C], bf, name="wq_sb")
            wk_sb = pp.tile([P, NCT, C], bf, name="wk_sb")
            wv_sb = pp.tile([P, NCT, C], bf, name="wv_sb")
            wo_sb = pp.tile([P, NCT, C], bf, name="wo_sb")
            bq_sb = pp.tile([P, NCT], f32, name="bq_sb")
            bo_sb = pp.tile([P, NCT], f32, name="bo_sb")
            ev_sb = pp.tile([P, NCT, P], bf, name="ev_sb")
            oh_sb = pp.tile([8, NCT, P], bf, name="oh_sb")
            cor1 = pp.tile([P, H, 5], bf, name="cor1")
            cor2 = pp.tile([P, H, 3], bf, name="cor2")
            am_sb = pp.tile([P, IB, P], bf, name="am_sb")

            q_sb = pp.tile([P, NCT, TQ], bf, name="q_sb")
            k_sb = pp.tile([P, NCT, T], bf, name="k_sb")
            vT_sb = pp.tile([P, 8, C], bf, name="vT_sb")
            eT_sb = pp.tile([P, H, 8, IB, P], bf, name="eT_sb")
            zall = pp.tile([P, IB, H], f32, name="zall")
            zrec = pp.tile([P, IB, H], bf, name="zrec")
            recz = pp.tile([8, TQ], bf, name="recz")
            rzb = pp.tile([P, NCT, TQ], f32, name="rzb")
            abg = pp.tile([P, IB, H, 16], bf, name="abg")
            abT_all = pp.tile([P, IB, P], bf, name="abT_all")
            bandm = pp.tile([P, NCT, TQ], f32, name="bandm")
            ctxm = pp.tile([P, 2, TQ], f32, name="ctxm")
            ctxn = pp.tile([P, NCT, TQ], bf, name="ctxn")

            # ---- input loads (use order; SP queue) --------------------
            nc.sync.dma_start(wq_sb[:], wqT[:])
            nc.sync.dma_start(xq_sb[:], xq[:])
            nc.sync.dma_start(bq_sb[:], bq8[:])
            nc.sync.dma_start(wk_sb[:], wkT[:])
            nc.sync.dma_start(xk_sb[:], xk[:])
            nc.sync.dma_start(cor1[:], cor1d[:])
            nc.sync.dma_start(cor2[:], cor2d[:])
            nc.sync.dma_start(wv_sb[:], wvT[:])
            nc.sync.dma_start(xv_sb[:], xv[:])
            nc.sync.dma_start(am_sb[:], amask[:])
            nc.sync.dma_start(ev_sb[:], evpr[:])
            nc.sync.dma_start(oh_sb[:], oneh[:])
            nc.sync.dma_start(wo_sb[:], woT[:])
            nc.sync.dma_start(bo_sb[:], bod[:])

            idbf = pp.tile([P, P], bf, name="idbf")
            make_identity(nc, idbf[:])
            nc.vector.memset(abg[:], 0.0)

            # ---- projection chains ------------------------------------
            def q_proj(co):
                pq = psC.tile([P, TQ], f32, name="pq")
                for ci in range(NCT):
                    nc.tensor.matmul(pq, wq_sb[:, ci, co * P:(co + 1) * P],
                                     xq_sb[:, ci, :], start=(ci == 0), stop=(ci == 3))
                nc.vector.tensor_scalar_add(q_sb[:, co, :], pq, bq_sb[:, co:co + 1])

            def k_proj_half(co, nh):
                # one j-half of the K projection for channel block co
                pk = psC.tile([P, TQ], f32, name="pq")
                ns = slice(nh * 512, nh * 512 + 512)
                for ci in range(NCT):
                    nc.tensor.matmul(pk, wk_sb[:, ci, co * P:(co + 1) * P],
                                     xk_sb[:, ci, ns],
                                     start=(ci == 0), stop=(ci == 3))
                nc.vector.tensor_copy(k_sb[:, co, ns], pk)

            def v_proj(tb):
                pv = psC.tile([P, TQ], f32, name="pq")
                for ci in range(NCT):
                    nc.tensor.matmul(pv, xv_sb[:, ci, tb * P:(tb + 1) * P],
                                     wv_sb[:, ci, :], start=(ci == 0), stop=(ci == 3))
                nc.vector.tensor_copy(vT_sb[:, tb, :], pv)

            pcs = {}

            def ctx_chunk(a, hh, jbs):
                # 4 accumulation matmuls into pc(a); creates the tile on
                # first use, stops on the very last (hh==1, jb==7)
                if a not in pcs:
                    pcs[a] = psC.tile([P, TQ], f32, name="pq")
                pc = pcs[a]
                h = 2 * a + hh
                for jb in jbs:
                    nc.tensor.matmul(
                        pc[hh * D:hh * D + D, :],
                        vT_sb[:, jb, h * D:h * D + D],
                        eT_sb[:, h, jb, :, :],
                        start=(jb == 0),
                        stop=(a <= 1 and jb == 7),
                        tile_position=(0, hh * D),
                        skip_group_check=True,
                    )
                if a <= 1 and hh == 1 and jbs[-1] == 7:
                    # spill to SBUF so the psC buffer recycles for a+2;
                    # a>=2 tiles stay open: band matmuls stop them
                    nc.vector.tensor_copy(ctxm[:, a, :], pc[:])

            # deferred PE work: (min_block_index, fn), drained in FIFO
            # order once the block counter passes min_block_index
            fill_q = [(0, lambda: k_proj_half(1, 0)),
                      (0, lambda: k_proj_half(1, 1)),
                      (0, lambda: q_proj(1))]
            for tb in range(8):
                fill_q.append((0, lambda t=tb: v_proj(t)))
            for co in range(2, NCT):
                fill_q.append((0, lambda c=co: k_proj_half(c, 0)))
                fill_q.append((0, lambda c=co: k_proj_half(c, 1)))
                fill_q.append((0, lambda c=co: q_proj(c)))

            def drain(bi):
                # pop at most one eligible unit per score block
                if fill_q and fill_q[0][0] <= bi:
                    fill_q.pop(0)[1]()

            # ---- pair-major scores / exp / transpose / stage ----------
            q_proj(0)
            k_proj_half(0, 0)
            k_proj_half(0, 1)


            bi = 0
            pending_g = []
            for a in range(NCT):
                # gathers of the previous pair: their stages are long done,
                # so they no longer head-of-line-block the gpsimd queue
                for g in pending_g:
                    g(nc.gpsimd)
                pending_g = []
                if a >= 1:
                    # ctx chunks of the previous pair: hold until the
                    # pair's transposes have certainly landed
                    for ci, (hh, j) in enumerate(
                            [(0, (0, 1, 2, 3)), (0, (4, 5, 6, 7)),
                             (1, (0, 1, 2, 3)), (1, (4, 5, 6, 7))]):
                        fill_q.append((a * 8 + 3 + ci,
                                       lambda aa=a - 1, hhh=hh, jj=j:
                                       ctx_chunk(aa, hhh, jj)))
                for ib in ((3, 0, 1, 2) if a == 3 else range(IB)):
                    isl = slice(ib * P, ib * P + P)
                    e_t = ep.tile([P, 2, T], bf, name="e_t")
                    for hh in range(2):
                        h = 2 * a + hh
                        hp = hh * D
                        sc = psS.tile([P, T], f32, name="mm")
                        for nh in range(2):
                            ns = slice(nh * 512, nh * 512 + 512)
                            nc.tensor.matmul(sc[:, ns],
                                             q_sb[hp:hp + D, a, isl],
                                             k_sb[hp:hp + D, a, ns],
                                             start=True, stop=True)
                        if ib == 0:
                            nc.vector.tensor_tensor(out=sc[0:32, 1019:1024],
                                                    in0=sc[0:32, 1019:1024],
                                                    in1=cor1[0:32, h, :],
                                                    op=add_op)
                        if ib == 3:
                            nc.vector.tensor_tensor(out=sc[96:128, 512:515],
                                                    in0=sc[96:128, 512:515],
                                                    in1=cor2[96:128, h, :],
                                                    op=add_op)
                        nc.scalar.activation(e_t[:, hh, :], sc[:], EXP,
                                             accum_out=zall[:, ib, h:h + 1])
                        drain(bi)
                        bi += 1
                    # one merged transpose for both heads of the pair
                    nc.sync.dma_start_transpose(
                        eT_sb[:, 2 * a:2 * a + 2, :, ib, :], e_t[:, :, :])

                    # stage band window to DRAM; last pair rides SP
                    hsl = slice(2 * a, 2 * a + 2)
                    st = stg[ib][a]
                    if ib == 0:
                        nc.gpsimd.dma_start(st[:, :, 4:W], e_t[:, :, 0:W - 4])
                        nc.gpsimd.dma_start(st[:, :, 0:4], e_t[:, :, T - 4:T])
                    else:
                        lo = ib * P - 4
                        nc.gpsimd.dma_start(st[:, :, :], e_t[:, :, lo:lo + W])
                    diag = AP(tensor=st[:].tensor, offset=0,
                              ap=[[2 * W + 1, P], [W, 2], [1, 9]])
                    pending_g.append(
                        lambda eng, d=diag, i=ib, hs=hsl:
                        eng.dma_start(abg[:, i, hs, 0:9], d))

            while fill_q:
                fill_q.pop(0)[1]()
            for g in pending_g:
                g(nc.sync)
            pending_g = []

            # ---- tail: Z transposes, band, ctx(3) ---------------------
            def z_tr(ib):
                with nc.allow_low_precision(reason="1/Z in bf16"):
                    nc.vector.reciprocal(zrec[:, ib, :], zall[:, ib, :])
                pzt = psT.tile([P, P], bf, name="tp")[0:8, :]
                nc.tensor.transpose(pzt, zrec[:, ib, :], idbf[:])
                nc.vector.tensor_copy(recz[:, ib * P:(ib + 1) * P], pzt)

            def ab_tr(ib):
                abm = ep.tile([P, P], bf, name="abm")
                nc.vector.tensor_tensor(
                    out=abm[:],
                    in0=abg[:, ib, :, :].rearrange("p h x -> p (h x)"),
                    in1=am_sb[:, ib, :], op=mul_op)
                pabt = psT.tile([P, P], bf, name="tp")
                nc.tensor.transpose(pabt[:], abm[:], idbf[:])
                nc.vector.tensor_copy(abT_all[:, ib, :], pabt[:])

            z_tr(3)
            z_tr(0)
            ab_tr(3)
            z_tr(1)
            ab_tr(0)
            z_tr(2)
            ctx_chunk(3, 0, (0, 1, 2, 3))
            ctx_chunk(3, 0, (4, 5, 6, 7))
            ab_tr(1)
            ctx_chunk(3, 1, (0, 1, 2, 3))
            ab_tr(2)
            ctx_chunk(3, 1, (4, 5, 6, 7))

            # 1/Z row broadcast -> [128, TQ] per pair (PE, then SBUF)
            for a in range(NCT):
                pz = psT.tile([P, TQ], f32, name="tp")
                nc.tensor.matmul(pz[:], oh_sb[:, a, :], recz[:],
                                 start=True, stop=True)
                nc.vector.tensor_copy(rzb[:, a, :], pz[:])

            # band matmuls: pairs 0/1 via transient psum + SBUF spill,
            # pairs 2/3 accumulate into the still-open ctx psum
            for a in (2, 3, 0, 1):
                if a >= 2:
                    for ib in range(IB):
                        nc.tensor.matmul(pcs[a][:, ib * P:(ib + 1) * P],
                                         ev_sb[:, a, :], abT_all[:, ib, :],
                                         start=False, stop=True,
                                         skip_group_check=True)
                else:
                    pba = psT.tile([P, TQ], f32, name="tp")
                    for ib in range(IB):
                        nc.tensor.matmul(pba[:, ib * P:(ib + 1) * P],
                                         ev_sb[:, a, :], abT_all[:, ib, :],
                                         start=True, stop=True,
                                         skip_group_check=True)
                    nc.vector.tensor_copy(bandm[:, a, :], pba[:])

            # ---- normalize ------------------------------------------
            for a in (2, 3, 0, 1):
                if a >= 2:
                    nc.vector.tensor_tensor(out=ctxn[:, a, :],
                                            in0=pcs[a][:],
                                            in1=rzb[:, a, :], op=mul_op)
                else:
                    csum = ep.tile([P, TQ], f32, name="csum")
                    nc.vector.tensor_tensor(out=csum[:], in0=ctxm[:, a, :],
                                            in1=bandm[:, a, :], op=add_op)
                    nc.vector.tensor_tensor(out=ctxn[:, a, :], in0=csum[:],
                                            in1=rzb[:, a, :], op=mul_op)

            # ---- output projection (+bias via Act copy) --------------
            for co in range(NCT):
                po = psS.tile([P, T], f32, name="mm")[:, 0:TQ]
                for ci in range(NCT):
                    nc.tensor.matmul(po, wo_sb[:, ci, co * P:(co + 1) * P],
                                     ctxn[:, ci, :], start=(ci == 0), stop=(ci == 3))
                o_sb = ep.tile([P, TQ], f32, name="o_sb")
                nc.scalar.activation(o_sb[:], po, IDENT,
                                     bias=bo_sb[:, co:co + 1])
                nc.sync.dma_start(Out[co], o_sb[:])

            if DBG:
                nc.sync.dma_start(DQ[:], q_sb[:])
                nc.sync.dma_start(DK[:], k_sb[:])
                nc.sync.dma_start(DV[:], vT_sb[:])
                nc.sync.dma_start(DZ[:], recz[:])
                nc.sync.dma_start(DCT[:], ctxn[:])
                nc.sync.dma_start(DBM[:], bandm[:])
                nc.sync.dma_start(DAB[:], abT_all[:])
                nc.sync.dma_start(DET[:], eT_sb[:])

    nc.compile()
    return nc


def _prep(inputs):
    x_q = np.asarray(inputs["x_q"], np.float32)
    x_k = np.asarray(inputs["x_k"], np.float32)
    x_v = np.asarray(inputs["x_v"], np.float32)
    Wq = np.asarray(inputs["Wq"], np.float32)
    Wk = np.asarray(inputs["Wk"], np.float32)
    Wv = np.asarray(inputs["Wv"], np.float32)
    Wo = np.asarray(inputs["Wo"], np.float32)
    bq = np.asarray(inputs["bq"], np.float32)
    bo = np.asarray(inputs["bo"], np.float32)
    bv = np.asarray(inputs["bv"], np.float32)
    ek = np.asarray(inputs["emb_rel_k"], np.float32)
    ev = np.asarray(inputs["emb_rel_v"], np.float32)

    def ctile(a):  # (C, X) -> (P, NCT, X) partition-first
        return np.ascontiguousarray(a.reshape(NCT, P, -1).transpose(1, 0, 2))

    wqT = ctile(Wq.T * 0.125).astype(BF16)
    wkT = ctile(Wk.T).astype(BF16)
    wvT = ctile(Wv.T).astype(BF16)
    woT = ctile(Wo.T).astype(BF16)
    bq8 = np.ascontiguousarray((bq * 0.125).reshape(NCT, P).T).astype(np.float32)
    bo_eff = bo + Wo @ bv
    bod = np.ascontiguousarray(bo_eff.reshape(NCT, P).T).astype(np.float32)

    evpr = np.zeros((NCT, P, P), np.float32)
    for a in range(NCT):
        for hh in range(2):
            h = 2 * a + hh
            for mt in range(9):
                evpr[a, 32 * a + 16 * hh + mt, hh * D:(hh + 1) * D] = ev[h, mt]
    evpr = np.ascontiguousarray(evpr.transpose(1, 0, 2)).astype(BF16)

    oneh = np.zeros((NCT, 8, P), np.float32)
    for a in range(NCT):
        for cc in range(P):
            oneh[a, 2 * a + cc // D, cc] = 1.0
    oneh = np.ascontiguousarray(oneh.transpose(1, 0, 2)).astype(BF16)

    in_maps = []
    for core in range(8):
        b, half = core // 2, core % 2
        i0 = half * TQ
        cor1 = np.zeros((P, H, 5), np.float32)
        cor2 = np.zeros((P, H, 3), np.float32)
        if half == 0:
            qc = (Wq @ x_q[b][:, 0:5]) * 0.125 + (bq[:, None] * 0.125)
            for h in range(H):
                for p in range(5):
                    for c in range(p, 5):
                        cor1[p, h, c] = qc[h * D:(h + 1) * D, p] @ ek[h, c - p]
        else:
            qc = (Wq @ x_q[b][:, 1019:1023]) * 0.125 + (bq[:, None] * 0.125)
            for h in range(H):
                for p in (125, 126, 127):
                    for c in range(3):
                        m = 133 + c - p
                        if 6 <= m <= 8:
                            cor2[p, h, c] = qc[h * D:(h + 1) * D, (895 + p) - 1019] @ ek[h, m]
        amask = np.zeros((IB, P, P), np.float32)
        for ib in range(IB):
            pvec = np.arange(P)
            for a in range(NCT):
                for mt in range(9):
                    j_g = i0 + ib * P + pvec + mt - 4
                    ok = ((j_g >= 0) & (j_g < T)).astype(np.float32)
                    amask[ib, :, a * 32 + 0 * 16 + mt] = ok
                    amask[ib, :, a * 32 + 1 * 16 + mt] = ok
        amask = np.ascontiguousarray(amask.transpose(1, 0, 2))
        in_maps.append({
            "xq": ctile(x_q[b][:, i0:i0 + TQ]).astype(BF16),
            "xk": ctile(np.roll(x_k[b], -i0, axis=1)).astype(BF16),
            "xv": ctile(np.roll(x_v[b], -i0, axis=1)).astype(BF16),
            "wqT": wqT, "wkT": wkT, "wvT": wvT, "woT": woT,
            "bq8": bq8, "bod": bod, "evpr": evpr, "oneh": oneh,
            "cor1d": cor1.astype(BF16), "cor2d": cor2.astype(BF16),
            "amask": amask.astype(BF16),
        })
    return in_maps


def kernel(**inputs):
    if "nc" not in _CACHE:
        _CACHE["nc"] = build_nc()
    nc = _CACHE["nc"]
    in_maps = _prep(inputs)
    res = run_bass_kernel_spmd(nc, in_maps, list(range(8)))
    out = np.zeros((4, C, T), np.float32)
    for core in range(8):
        b, half = core // 2, core % 2
        o = np.asarray(res.results[core]["Out"]).reshape(C, TQ)
        out[b][:, half * TQ:(half + 1) * TQ] = o
    return out
